# revision 1
# baseline (speedup 1.0000x reference)
"""Trainium2 Bass kernel for nn_AttentionModel (B=8, S=2048, D=1024).

Strategy: data-parallel over batch — core b computes batch b entirely
locally (no collectives).

Per-core dataflow (all matmuls bf16 on TensorE, fp32 PSUM accumulate):
  inputs (host-prepped layouts):
    x1t/x2t/x3t [D, S]  = plmsN[b].T          (contraction dim on partitions)
    wqt/wkt/wvt [D, D]  = W.T                 (in-dim on partitions)
    bqp/bkp     [128, 8] = bias.reshape(8,128).T   (per-partition columns)
    bvr         [D]      = bv                  (broadcast along free dim)
    maskp       [128,16] = key-mask columns (0 / -30000), kth tile col
  phase A: QT[d,s] = wqt.T @ x1t  (+bq), KT[d,s] likewise,
           V[s,d]  = x3t.T @ wvt  (+bv), plus a ones column V[:,1024]=1
  phase B per 512-wide q-chunk:
    S^T[k,q] tiles = KT.T @ QT   -> exp(scale*x + mask_k) -> bf16 expS
    per 128-query tile: O' = expS.T @ V  (d cols 0:512, 512:1024, and the
    ones column gives the softmax denominator in O'[:,1024])
    out[q,:] = O'[q,:]/denom[q] + V[q,:]   -> DMA out
"""

import numpy as np

B, S, D = 8, 2048, 1024
P = 128
NQ = 512                 # moving free dim (q-chunk and n-chunk width)
N_QCHUNK = S // NQ       # 4
KT_TILES = S // P        # 16 key tiles
DT_TILES = D // P        # 8 d tiles
ND_CHUNK = D // NQ       # 2 d chunks for V / output
SCALE = 1.0 / float(np.sqrt(D))
NEG_MASK = -30000.0


def _apply_tile_patch():
    """This walrus build allows at most ONE semaphore wait on the tail
    CTRL/Drain instruction; Tile's kernel-tail drain carries one wait per
    touched logical proc. Spread them over multiple drains."""
    import copy

    from concourse import tile as _tile
    from concourse.vector_clock import ScopedClock as _ScopedClock

    if getattr(_tile.TileContext, "_drain_patch_applied", False):
        return

    def _patched(self, tick_clock, wait_clock):
        nc = self.nc
        drain_inst = nc.sync.drain()
        wait_clock.add_sem_waits(
            drain_inst.ins, _ScopedClock({None: tick_clock.global_clock})
        )
        mi = drain_inst.ins
        si = mi.sync_info
        waits = list(si.on_wait) if (si is not None and si.on_wait) else []
        if len(waits) > 1:
            si.on_wait = waits[:1]
            mi.sync_info = si
            for i in range(1, len(waits)):
                extra = nc.sync.drain()
                esi = copy.copy(si)
                esi.on_wait = [waits[i]]
                esi.on_update = []
                extra.ins.sync_info = esi

        nc.all_engine_barrier()
        assert self.sems is not None
        popped = nc._tile_sem_poison_stack.pop()
        assert popped is self._sem_poison
        nc.clear_and_free_semaphores(list(self.sems.allocated().values()))
        nc.all_engine_barrier()

    _tile.TileContext._drain_and_barrier = _patched
    _tile.TileContext._drain_patch_applied = True


def _split_excess_waits(nc, max_waits=1):
    """This walrus build rejects instructions carrying more than one
    semaphore wait ("Too many sync wait commands"). Hoist extra waits onto
    same-engine NoOp carriers inserted right before the instruction."""
    from concourse import mybir

    n_split = 0
    for f in nc.m.functions:
        for blk in f.blocks:
            insts = list(blk.instructions)
            out = []
            changed = False
            for inst in insts:
                si = inst.sync_info
                waits = list(si.on_wait) if (si is not None and si.on_wait) else []
                if len(waits) > max_waits:
                    head, tail = waits[:-max_waits], waits[-max_waits:]
                    for i in range(0, len(head), max_waits):
                        carrier = mybir.InstNoOp(
                            name=nc.get_next_instruction_name(),
                            engine=inst.engine,
                            ins=[],
                            outs=[],
                            sync_info=mybir.SyncInfo(
                                on_wait=head[i : i + max_waits], on_update=[]
                            ),
                        )
                        out.append(carrier)
                    si.on_wait = tail
                    inst.sync_info = si
                    changed = True
                    n_split += 1
                out.append(inst)
            if changed:
                blk.instructions = out
    return n_split


def _install_neff_cache():
    """walrus compile of this kernel takes ~10 min; cache the NEFF keyed on
    the BIR json hash so repeat runs (same graph) skip it."""
    import hashlib
    import os
    import shutil

    from concourse import bass2jax, bass_utils

    if getattr(bass_utils, "_neff_cache_installed", False):
        return
    orig = bass_utils.compile_bir_kernel

    def cached(bir_json, tmpdir, neff_name="file.neff"):
        h = hashlib.sha256(bytes(bir_json)).hexdigest()[:32]
        cdir = os.path.expanduser("~/.bass-neff-cache")
        os.makedirs(cdir, exist_ok=True)
        cpath = os.path.join(cdir, h + ".neff")
        if os.path.exists(cpath):
            dst = os.path.join(tmpdir, neff_name)
            shutil.copyfile(cpath, dst)
            return dst
        p = orig(bir_json, tmpdir, neff_name)
        try:
            shutil.copyfile(p, cpath)
        except OSError:
            pass
        return p

    bass_utils.compile_bir_kernel = cached
    bass2jax.compile_bir_kernel = cached
    bass_utils._neff_cache_installed = True


def build_nc(split_waits=True):
    """Build the per-core Bass graph (SPMD: same graph on all 8 cores)."""
    import concourse.bass as bass
    import concourse.tile as tile
    from concourse import mybir

    _apply_tile_patch()

    f32 = mybir.dt.float32
    bf16 = mybir.dt.bfloat16
    AF = mybir.ActivationFunctionType

    nc = bass.Bass()

    x1t = nc.dram_tensor("x1t", [D, S], bf16, kind="ExternalInput")
    x2t = nc.dram_tensor("x2t", [D, S], bf16, kind="ExternalInput")
    x3t = nc.dram_tensor("x3t", [D, S], bf16, kind="ExternalInput")
    wqt = nc.dram_tensor("wqt", [D, D], bf16, kind="ExternalInput")
    wkt = nc.dram_tensor("wkt", [D, D], bf16, kind="ExternalInput")
    wvt = nc.dram_tensor("wvt", [D, D], bf16, kind="ExternalInput")
    bqp = nc.dram_tensor("bqp", [P, DT_TILES], f32, kind="ExternalInput")
    bkp = nc.dram_tensor("bkp", [P, DT_TILES], f32, kind="ExternalInput")
    bvr = nc.dram_tensor("bvr", [D], f32, kind="ExternalInput")
    maskp = nc.dram_tensor("maskp", [P, KT_TILES], f32, kind="ExternalInput")
    out = nc.dram_tensor("out", [S, D], f32, kind="ExternalOutput")

    with tile.TileContext(nc) as tc:
        with (
            tc.tile_pool(name="persist", bufs=1) as persist,
            tc.tile_pool(name="consts", bufs=1) as consts,
        ):
            # Persistent SBUF tensors.
            qt_sb = [
                persist.tile([P, S], bf16, tag=f"qt{i}", name=f"qt{i}")
                for i in range(DT_TILES)
            ]
            kt_sb = [
                persist.tile([P, S], bf16, tag=f"kt{i}", name=f"kt{i}")
                for i in range(DT_TILES)
            ]
            # V tiles have a trailing ones column at free index D.
            v_sb = [
                persist.tile([P, D + 1], bf16, tag=f"v{i}", name=f"v{i}")
                for i in range(KT_TILES)
            ]

            bq_sb = consts.tile([P, DT_TILES], f32, tag="bq")
            bk_sb = consts.tile([P, DT_TILES], f32, tag="bk")
            mask_sb = consts.tile([P, KT_TILES], f32, tag="mask")
            bv_sb = consts.tile([P, D], f32, tag="bv")
            nc.sync.dma_start(out=bq_sb[:], in_=bqp[:, :])
            nc.sync.dma_start(out=bk_sb[:], in_=bkp[:, :])
            nc.sync.dma_start(out=mask_sb[:], in_=maskp[:, :])
            bvr_ap = bvr[:]
            bv_bcast = bass.AP(
                tensor=bvr_ap.tensor, offset=bvr_ap.offset, ap=[[0, P], [1, D]]
            )
            nc.sync.dma_start(out=bv_sb[:], in_=bv_bcast)

            # ---------------- Phase A: projections ----------------
            # x streamed in [128, S/2] half-tiles (tag xt, 14 slots: 8 live
            # + 6 prefetch so the next projection's loads hide under the
            # current one's matmuls); weights per projection (8 live + 2
            # prefetch). Inner loops keep each lhsT loaded for 2 matmuls.
            SH = S // 2  # 1024, s-half width
            with (
                tc.tile_pool(name="xw", bufs=18) as xw_pool,
                tc.tile_pool(name="wts", bufs=12) as w_pool,
                tc.tile_pool(name="psA", bufs=4, space="PSUM") as psA,
            ):
                def load_w(src_t):
                    tiles = []
                    for ii in range(DT_TILES):
                        t = w_pool.tile([P, D], bf16, tag="wt", name=f"wt{ii}")
                        nc.sync.dma_start(
                            out=t[:], in_=src_t[ii * P : (ii + 1) * P, :]
                        )
                        tiles.append(t)
                    return tiles

                def load_x_half(src_t, h):
                    tiles = []
                    for ii in range(DT_TILES):
                        t = xw_pool.tile([P, SH], bf16, tag="xt", name=f"xt{ii}")
                        nc.sync.dma_start(
                            out=t[:],
                            in_=src_t[ii * P : (ii + 1) * P, h * SH : (h + 1) * SH],
                        )
                        tiles.append(t)
                    return tiles

                def load_first_interleaved(xsrc, wsrc):
                    # Interleave x/w DMAs so the first matmul's operands
                    # don't queue behind the whole weight matrix.
                    w_tiles, x_tiles = [], []
                    for ii in range(DT_TILES):
                        xt_ = xw_pool.tile([P, SH], bf16, tag="xt", name=f"xt{ii}")
                        nc.sync.dma_start(
                            out=xt_[:], in_=xsrc[ii * P : (ii + 1) * P, 0:SH]
                        )
                        x_tiles.append(xt_)
                        wt_ = w_pool.tile([P, D], bf16, tag="wt", name=f"wt{ii}")
                        nc.sync.dma_start(
                            out=wt_[:], in_=wsrc[ii * P : (ii + 1) * P, :]
                        )
                        w_tiles.append(wt_)
                    return x_tiles, w_tiles

                # --- Q and K projections: out tiles are [d_out, s] ---
                for pi, (xsrc, wsrc, dst, bias_sb) in enumerate((
                    (x1t, wqt, qt_sb, bq_sb),
                    (x2t, wkt, kt_sb, bk_sb),
                )):
                    first_x = None
                    if pi == 0:
                        first_x, w_tiles = load_first_interleaved(xsrc, wsrc)
                    else:
                        w_tiles = load_w(wsrc)
                    for h in range(2):
                        x_tiles = (
                            first_x if (h == 0 and first_x is not None)
                            else load_x_half(xsrc, h)
                        )
                        for di in range(DT_TILES):
                            ps2 = [
                                psA.tile([P, NQ], f32, tag="psA", name="psA_t")
                                for _ in range(2)
                            ]
                            for ii in range(DT_TILES):
                                lhsT = w_tiles[ii][:, di * P : (di + 1) * P]
                                for j in range(2):
                                    nc.tensor.matmul(
                                        ps2[j][:],
                                        lhsT=lhsT,
                                        rhs=x_tiles[ii][:, j * NQ : (j + 1) * NQ],
                                        start=(ii == 0),
                                        stop=(ii == DT_TILES - 1),
                                    )
                            for j in range(2):
                                sc = h * 2 + j
                                nc.scalar.activation(
                                    out=dst[di][:, sc * NQ : (sc + 1) * NQ],
                                    in_=ps2[j][:],
                                    func=AF.Identity,
                                    bias=bias_sb[:, di : di + 1],
                                    scale=1.0,
                                )

                # --- V projection: out tiles are [s, d] ---
                w_tiles = load_w(wvt)
                for h in range(2):
                    x_tiles = load_x_half(x3t, h)
                    for sl in range(KT_TILES // 2):
                        si = h * (KT_TILES // 2) + sl
                        ps2 = [
                            psA.tile([P, NQ], f32, tag="psA", name="psA_t")
                            for _ in range(2)
                        ]
                        for ii in range(DT_TILES):
                            lhsT = x_tiles[ii][:, sl * P : (sl + 1) * P]
                            for dc in range(ND_CHUNK):
                                nc.tensor.matmul(
                                    ps2[dc][:],
                                    lhsT=lhsT,
                                    rhs=w_tiles[ii][:, dc * NQ : (dc + 1) * NQ],
                                    start=(ii == 0),
                                    stop=(ii == DT_TILES - 1),
                                )
                        for dc in range(ND_CHUNK):
                            # psum + bv (varies along free dim) -> sbuf bf16
                            nc.vector.tensor_add(
                                out=v_sb[si][:, dc * NQ : (dc + 1) * NQ],
                                in0=ps2[dc][:],
                                in1=bv_sb[:, dc * NQ : (dc + 1) * NQ],
                            )
                        # ones column for the softmax denominator
                        nc.vector.memset(v_sb[si][:, D : D + 1], 1.0)

            # ---------------- Phase B: attention ----------------
            with (
                tc.tile_pool(name="es", bufs=KT_TILES + 8) as es_pool,
                tc.tile_pool(name="outp", bufs=3) as out_pool,
                tc.tile_pool(name="recp", bufs=4) as rec_pool,
                tc.tile_pool(name="psS", bufs=3, space="PSUM") as psS,
                tc.tile_pool(name="psO", bufs=2, space="PSUM") as psO,
                tc.tile_pool(name="psD", bufs=1, space="PSUM") as psD,
            ):
                for qc in range(N_QCHUNK):
                    # scores^T tiles for this q-chunk: [k 128, q 512]
                    es_tiles = []
                    for kt in range(KT_TILES):
                        ps = psS.tile([P, NQ], f32, tag="psS", name="psS_t")
                        for di in range(DT_TILES):
                            nc.tensor.matmul(
                                ps[:],
                                lhsT=kt_sb[di][:, kt * P : (kt + 1) * P],
                                rhs=qt_sb[di][:, qc * NQ : (qc + 1) * NQ],
                                start=(di == 0),
                                stop=(di == DT_TILES - 1),
                            )
                        es = es_pool.tile([P, NQ], bf16, tag="es", name="es_t")
                        nc.scalar.activation(
                            out=es[:],
                            in_=ps[:],
                            func=AF.Exp,
                            bias=mask_sb[:, kt : kt + 1],
                            scale=SCALE,
                        )
                        es_tiles.append(es)

                    # attn @ V for the 4 query tiles of this chunk
                    for qi in range(NQ // P):
                        qg = qc * (NQ // P) + qi
                        po = [
                            psO.tile([P, NQ], f32, tag=f"o{dc}", name=f"psO_{dc}")
                            for dc in range(ND_CHUNK)
                        ]
                        pd = psD.tile([P, 1], f32, tag="pd", name="pd_t")
                        for kt in range(KT_TILES):
                            lhsT = es_tiles[kt][:, qi * P : (qi + 1) * P]
                            st = (kt == 0)
                            sp = (kt == KT_TILES - 1)
                            for dc in range(ND_CHUNK):
                                nc.tensor.matmul(
                                    po[dc][:],
                                    lhsT=lhsT,
                                    rhs=v_sb[kt][:, dc * NQ : (dc + 1) * NQ],
                                    start=st,
                                    stop=sp,
                                )
                            nc.tensor.matmul(
                                pd[:],
                                lhsT=lhsT,
                                rhs=v_sb[kt][:, D : D + 1],
                                start=st,
                                stop=sp,
                            )
                        rec = rec_pool.tile([P, 1], f32, tag="rec", name="rec_t")
                        nc.vector.reciprocal(out=rec[:], in_=pd[:])
                        o_sb = out_pool.tile([P, D], f32, tag="ot", name="ot_t")
                        for dc in range(ND_CHUNK):
                            sl = slice(dc * NQ, (dc + 1) * NQ)
                            nc.vector.tensor_scalar_mul(
                                out=o_sb[:, sl], in0=po[dc][:], scalar1=rec[:],
                            )
                            nc.vector.tensor_add(
                                out=o_sb[:, sl],
                                in0=o_sb[:, sl],
                                in1=v_sb[qg][:, sl],
                            )
                            nc.sync.dma_start(
                                out=out[qg * P : (qg + 1) * P, sl],
                                in_=o_sb[:, sl],
                            )

    if split_waits:
        _split_excess_waits(nc)
    return nc


def _prep_inputs(plms1, plms2, plms3, seqlengths, Wq, bq, Wk, bk, Wv, bv):
    """Host-side shard + layout prep. Returns in_maps for 8 cores."""
    import ml_dtypes

    bf = ml_dtypes.bfloat16
    f32 = np.float32

    def t_bf(a):  # [S, D] -> [D, S] bf16 contiguous
        return np.ascontiguousarray(np.asarray(a, f32).T).astype(bf)

    wqt = np.ascontiguousarray(np.asarray(Wq, f32).T).astype(bf)
    wkt = np.ascontiguousarray(np.asarray(Wk, f32).T).astype(bf)
    wvt = np.ascontiguousarray(np.asarray(Wv, f32).T).astype(bf)
    bqp = np.ascontiguousarray(np.asarray(bq, f32).reshape(DT_TILES, P).T)
    bkp = np.ascontiguousarray(np.asarray(bk, f32).reshape(DT_TILES, P).T)
    bvr = np.asarray(bv, f32)
    seqlengths = np.asarray(seqlengths)

    in_maps = []
    ar = np.arange(S)
    for b in range(B):
        mask = np.where(ar < int(seqlengths[b]), 0.0, NEG_MASK).astype(f32)
        maskp = np.ascontiguousarray(mask.reshape(KT_TILES, P).T)
        in_maps.append(
            {
                "x1t": t_bf(np.asarray(plms1)[b]),
                "x2t": t_bf(np.asarray(plms2)[b]),
                "x3t": t_bf(np.asarray(plms3)[b]),
                "wqt": wqt,
                "wkt": wkt,
                "wvt": wvt,
                "bqp": bqp,
                "bkp": bkp,
                "bvr": bvr,
                "maskp": maskp,
            }
        )
    return in_maps


def kernel(**inputs) -> np.ndarray:
    from concourse.bass_utils import run_bass_kernel_spmd

    _install_neff_cache()

    in_maps = _prep_inputs(
        inputs["plms1"], inputs["plms2"], inputs["plms3"], inputs["seqlengths"],
        inputs["Wq"], inputs["bq"], inputs["Wk"], inputs["bk"],
        inputs["Wv"], inputs["bv"],
    )
    nc = build_nc()
    res = run_bass_kernel_spmd(nc, in_maps, core_ids=list(range(B)))
    return np.stack(
        [np.asarray(res.results[i]["out"], np.float32) for i in range(B)]
    )



# revision 2
# speedup vs baseline: 1.8618x; 1.8618x over previous
"""Trainium2 Bass kernel for nn_AttentionModel (B=8, S=2048, D=1024).

Strategy: data-parallel over batch — core b computes batch b entirely
locally (no collectives).

Math restructure (softmax is invariant to per-row constants):
  scores[q,k] = Q[q]·K[k] = x1_q (Wq^T Wk) x2_k^T + alpha_q + beta_k + c
  alpha_q + c drop out in softmax;  M2 = Wq^T Wk is precomputed on host,
  beta_k = x2_k · (Wk^T bq) folds into the per-key exp bias column.
  So the K projection vanishes and the Q projection becomes H1 = x1 @ M2.

Per-core dataflow (fp8e4 DoubleRow matmuls, fp32 PSUM accumulate):
  phase A: H1^T tiles = M2s.T @ x1^T   (DR fp8, M2s = 32*M2 in fp8)
           V[s,d] = x3t.T @ wvt (+bv)  (bf16; the +V residual dominates
           the output so it needs bf16 accuracy) -> vres bf16 + v2 fp8
           (DR-paired k rows, with a ones column for the denominator)
  phase B per 512-wide q-chunk:
    scoresT[k,q] tiles = x2p.T @ h1t   (DR fp8)
    es = exp(SCALE/32 * psum + bias_k) (bias = SCALE*beta + key mask)
    per 128-query tile: po = es.T @ v2 (DR), pd = es.T @ ones
    out[q,:] = po[q,:]/pd[q] + vres[q,:]  -> DMA out

DoubleRow layout: operands are 3D APs [128, 2, n] — partition p, pair
half i covers contraction index 256*j + 128*i + p for chunk j.
"""

import numpy as np

B, S, D = 8, 2048, 1024
P = 128
NQ = 512                 # moving free dim per matmul output
N_QCHUNK = S // NQ       # 4
KT_TILES = S // P        # 16 key tiles of 128
KT2 = KT_TILES // 2      # 8 DR key chunks of 256
DJ = D // 256            # 4 DR contraction chunks over d
VW = 1040                # v2 per-half width: 1024 d cols + ones col + pad
SCALE = 1.0 / float(np.sqrt(D))
M2_SCALE = 32.0
NEG_MASK = -30000.0


def _apply_tile_patch():
    """This walrus build allows at most ONE semaphore wait on the tail
    CTRL/Drain instruction; Tile's kernel-tail drain carries one wait per
    touched logical proc. Spread them over multiple drains."""
    import copy

    from concourse import tile as _tile
    from concourse.vector_clock import ScopedClock as _ScopedClock

    if getattr(_tile.TileContext, "_drain_patch_applied", False):
        return

    def _patched(self, tick_clock, wait_clock):
        nc = self.nc
        drain_inst = nc.sync.drain()
        wait_clock.add_sem_waits(
            drain_inst.ins, _ScopedClock({None: tick_clock.global_clock})
        )
        mi = drain_inst.ins
        si = mi.sync_info
        waits = list(si.on_wait) if (si is not None and si.on_wait) else []
        if len(waits) > 1:
            si.on_wait = waits[:1]
            mi.sync_info = si
            for i in range(1, len(waits)):
                extra = nc.sync.drain()
                esi = copy.copy(si)
                esi.on_wait = [waits[i]]
                esi.on_update = []
                extra.ins.sync_info = esi

        nc.all_engine_barrier()
        assert self.sems is not None
        popped = nc._tile_sem_poison_stack.pop()
        assert popped is self._sem_poison
        nc.clear_and_free_semaphores(list(self.sems.allocated().values()))
        nc.all_engine_barrier()

    _tile.TileContext._drain_and_barrier = _patched
    _tile.TileContext._drain_patch_applied = True


def _split_excess_waits(nc, max_waits=1):
    """This walrus build rejects instructions carrying more than one
    semaphore wait ("Too many sync wait commands"). Hoist extra waits onto
    same-engine NoOp carriers inserted right before the instruction."""
    from concourse import mybir

    n_split = 0
    for f in nc.m.functions:
        for blk in f.blocks:
            insts = list(blk.instructions)
            out = []
            changed = False
            for inst in insts:
                si = inst.sync_info
                waits = list(si.on_wait) if (si is not None and si.on_wait) else []
                if len(waits) > max_waits:
                    head, tail = waits[:-max_waits], waits[-max_waits:]
                    for i in range(0, len(head), max_waits):
                        carrier = mybir.InstNoOp(
                            name=nc.get_next_instruction_name(),
                            engine=inst.engine,
                            ins=[],
                            outs=[],
                            sync_info=mybir.SyncInfo(
                                on_wait=head[i : i + max_waits], on_update=[]
                            ),
                        )
                        out.append(carrier)
                    si.on_wait = tail
                    inst.sync_info = si
                    changed = True
                    n_split += 1
                out.append(inst)
            if changed:
                blk.instructions = out
    return n_split


def _install_neff_cache():
    """walrus compile of this kernel takes ~10 min; cache the NEFF keyed on
    the BIR json hash so repeat runs (same graph) skip it."""
    import hashlib
    import os
    import shutil

    from concourse import bass2jax, bass_utils

    if getattr(bass_utils, "_neff_cache_installed", False):
        return
    orig = bass_utils.compile_bir_kernel

    def cached(bir_json, tmpdir, neff_name="file.neff"):
        h = hashlib.sha256(bytes(bir_json)).hexdigest()[:32]
        cdir = os.path.expanduser("~/.bass-neff-cache")
        os.makedirs(cdir, exist_ok=True)
        cpath = os.path.join(cdir, h + ".neff")
        if os.path.exists(cpath):
            dst = os.path.join(tmpdir, neff_name)
            shutil.copyfile(cpath, dst)
            return dst
        p = orig(bir_json, tmpdir, neff_name)
        try:
            shutil.copyfile(p, cpath)
        except OSError:
            pass
        return p

    bass_utils.compile_bir_kernel = cached
    bass2jax.compile_bir_kernel = cached
    bass_utils._neff_cache_installed = True


def _ap3(t_ap, pstride, col_off, half_step, n):
    """3D DoubleRow AP [128, 2, n] over an SBUF tile: partition stride
    pstride, halves half_step elements apart, n contiguous elements."""
    import concourse.bass as bass

    return bass.AP(
        tensor=t_ap.tensor,
        offset=t_ap.offset + col_off,
        ap=[[pstride, P], [half_step, 2], [1, n]],
    )


def build_nc(split_waits=True):
    """Build the per-core Bass graph (SPMD: same graph on all 8 cores)."""
    import concourse.bass as bass
    import concourse.tile as tile
    from concourse import mybir

    _apply_tile_patch()

    f32 = mybir.dt.float32
    bf16 = mybir.dt.bfloat16
    f8 = mybir.dt.float8e4
    AF = mybir.ActivationFunctionType
    DR = mybir.MatmulPerfMode.DoubleRow

    nc = bass.Bass()

    x1p = nc.dram_tensor("x1p", [DJ * P, 2 * S], f8, kind="ExternalInput")
    x2p = nc.dram_tensor("x2p", [DJ * P, 2 * S], f8, kind="ExternalInput")
    m2p = nc.dram_tensor("m2p", [DJ * P, 2 * D], f8, kind="ExternalInput")
    x3t = nc.dram_tensor("x3t", [D, S], bf16, kind="ExternalInput")
    wvt = nc.dram_tensor("wvt", [D, D], bf16, kind="ExternalInput")
    bvr = nc.dram_tensor("bvr", [D], f32, kind="ExternalInput")
    biasp = nc.dram_tensor("biasp", [P, KT_TILES], f32, kind="ExternalInput")
    out = nc.dram_tensor("out", [S, D], f32, kind="ExternalOutput")

    with tile.TileContext(nc) as tc:
        with (
            tc.tile_pool(name="persist", bufs=1) as persist,
            tc.tile_pool(name="consts", bufs=1) as consts,
        ):
            # Persistent SBUF tensors.
            h1t = [
                persist.tile([P, 2 * S], f8, tag=f"h1t{j}", name=f"h1t{j}")
                for j in range(DJ)
            ]
            x2sb = [
                persist.tile([P, 2 * S], f8, tag=f"x2s{j}", name=f"x2s{j}")
                for j in range(DJ)
            ]
            v2 = [
                persist.tile([P, 2 * VW], f8, tag=f"v2_{i}", name=f"v2_{i}")
                for i in range(KT2)
            ]
            vres = [
                persist.tile([P, D], bf16, tag=f"vr{i}", name=f"vr{i}")
                for i in range(KT_TILES)
            ]

            bias_sb = consts.tile([P, KT_TILES], f32, tag="bias")
            bv_sb = consts.tile([P, D], f32, tag="bv")
            nc.sync.dma_start(out=bias_sb[:], in_=biasp[:, :])
            bvr_ap = bvr[:]
            bv_bcast = bass.AP(
                tensor=bvr_ap.tensor, offset=bvr_ap.offset, ap=[[0, P], [1, D]]
            )
            nc.sync.dma_start(out=bv_sb[:], in_=bv_bcast)

            # ones columns of v2 (softmax denominator source)
            for i in range(KT2):
                for h in range(2):
                    nc.vector.memset(v2[i][:, h * VW + D : h * VW + D + 1], 1.0)

            # ---------------- Phase A: H1 projection + V ----------------
            with (
                tc.tile_pool(name="x1a", bufs=DJ) as x1_pool,
                tc.tile_pool(name="m2a", bufs=DJ) as m2_pool,
                tc.tile_pool(name="wva", bufs=8) as wv_pool,
                tc.tile_pool(name="x3a", bufs=8) as x3_pool,
                tc.tile_pool(name="psA", bufs=4, space="PSUM") as psA,
            ):
                # DMA order = consumption order: m2/x1 interleaved (H1
                # starts after ~1MB), then x2 (scores), then wv/x3 (V).
                m2sb, x1sb = [], []
                for j in range(DJ):
                    mt = m2_pool.tile([P, 2 * D], f8, tag="m2", name=f"m2_{j}")
                    nc.sync.dma_start(out=mt[:], in_=m2p[j * P : (j + 1) * P, :])
                    m2sb.append(mt)
                    xt = x1_pool.tile([P, 2 * S], f8, tag="x1", name=f"x1_{j}")
                    nc.sync.dma_start(out=xt[:], in_=x1p[j * P : (j + 1) * P, :])
                    x1sb.append(xt)
                wv_tiles, x3_tiles = [], []
                for i in range(8):
                    wt = wv_pool.tile([P, D], bf16, tag="wv", name=f"wv{i}")
                    nc.sync.dma_start(out=wt[:], in_=wvt[i * P : (i + 1) * P, :])
                    wv_tiles.append(wt)
                    x3t_ = x3_pool.tile([P, S], bf16, tag="x3", name=f"x3_{i}")
                    nc.sync.dma_start(out=x3t_[:], in_=x3t[i * P : (i + 1) * P, :])
                    x3_tiles.append(x3t_)
                for j in range(DJ):
                    nc.sync.dma_start(
                        out=x2sb[j][:], in_=x2p[j * P : (j + 1) * P, :]
                    )

                # --- H1^T[do, s] tiles = sum_dj M2s[dj].T @ x1T[dj] ---
                for sc in range(N_QCHUNK):
                    for do in range(D // P):
                        ps = psA.tile([P, NQ], f32, tag="psA", name="psA_t")
                        for dj in range(DJ):
                            nc.tensor.matmul(
                                ps[:],
                                lhsT=_ap3(m2sb[dj][:], 2 * D, do * P, D, P),
                                rhs=_ap3(x1sb[dj][:], 2 * S, sc * NQ, S, NQ),
                                start=(dj == 0),
                                stop=(dj == DJ - 1),
                                perf_mode=DR,
                            )
                        nc.scalar.activation(
                            out=h1t[do // 2][
                                :, (do % 2) * S + sc * NQ : (do % 2) * S + (sc + 1) * NQ
                            ],
                            in_=ps[:],
                            func=AF.Identity,
                            scale=1.0,
                        )

                # --- V[s, d] tiles = x3t.T @ wvt (+bv), bf16 ---
                for si in range(KT_TILES):
                    ps2 = [
                        psA.tile([P, NQ], f32, tag="psA", name="psA_t")
                        for _ in range(2)
                    ]
                    for ii in range(8):
                        lhsT = x3_tiles[ii][:, si * P : (si + 1) * P]
                        for dc in range(2):
                            nc.tensor.matmul(
                                ps2[dc][:],
                                lhsT=lhsT,
                                rhs=wv_tiles[ii][:, dc * NQ : (dc + 1) * NQ],
                                start=(ii == 0),
                                stop=(ii == 7),
                            )
                    for dc in range(2):
                        sl = slice(dc * NQ, (dc + 1) * NQ)
                        nc.vector.tensor_add(
                            out=vres[si][:, sl], in0=ps2[dc][:], in1=bv_sb[:, sl]
                        )
                        nc.scalar.activation(
                            out=v2[si // 2][
                                :, (si % 2) * VW + dc * NQ : (si % 2) * VW + (dc + 1) * NQ
                            ],
                            in_=vres[si][:, sl],
                            func=AF.Identity,
                            scale=1.0,
                        )

            # ---------------- Phase B: attention ----------------
            with (
                tc.tile_pool(name="es", bufs=KT2 + 2) as es_pool,
                tc.tile_pool(name="outp", bufs=4) as out_pool,
                tc.tile_pool(name="recp", bufs=4) as rec_pool,
                tc.tile_pool(name="psS", bufs=2, space="PSUM") as psS,
                tc.tile_pool(name="psO", bufs=4, space="PSUM") as psO,
                tc.tile_pool(name="psD", bufs=2, space="PSUM") as psD,
            ):
                for qc in range(N_QCHUNK):
                    # scoresT tiles [k 128, q 512] -> exp -> fp8 es pairs
                    es_tiles = []
                    for kt2 in range(KT2):
                        es = es_pool.tile([P, 2 * NQ], f8, tag="es", name="es_t")
                        for h in range(2):
                            kt = 2 * kt2 + h
                            ps = psS.tile([P, NQ], f32, tag="psS", name="psS_t")
                            for dj in range(DJ):
                                nc.tensor.matmul(
                                    ps[:],
                                    lhsT=_ap3(x2sb[dj][:], 2 * S, kt * P, S, P),
                                    rhs=_ap3(h1t[dj][:], 2 * S, qc * NQ, S, NQ),
                                    start=(dj == 0),
                                    stop=(dj == DJ - 1),
                                    perf_mode=DR,
                                )
                            nc.scalar.activation(
                                out=es[:, h * NQ : (h + 1) * NQ],
                                in_=ps[:],
                                func=AF.Exp,
                                bias=bias_sb[:, kt : kt + 1],
                                scale=SCALE / M2_SCALE,
                            )
                        es_tiles.append(es)

                    # attn @ V for the 4 query tiles of this chunk
                    for qi in range(NQ // P):
                        qg = qc * (NQ // P) + qi
                        po = [
                            psO.tile([P, NQ], f32, tag="po", name=f"psO_{dc}")
                            for dc in range(2)
                        ]
                        pd = psD.tile([P, 1], f32, tag="pd", name="pd_t")
                        for kt2 in range(KT2):
                            lhsT = _ap3(es_tiles[kt2][:], 2 * NQ, qi * P, NQ, P)
                            st = (kt2 == 0)
                            sp = (kt2 == KT2 - 1)
                            for dc in range(2):
                                nc.tensor.matmul(
                                    po[dc][:],
                                    lhsT=lhsT,
                                    rhs=_ap3(v2[kt2][:], 2 * VW, dc * NQ, VW, NQ),
                                    start=st,
                                    stop=sp,
                                    perf_mode=DR,
                                )
                            nc.tensor.matmul(
                                pd[:],
                                lhsT=lhsT,
                                rhs=_ap3(v2[kt2][:], 2 * VW, D, VW, 1),
                                start=st,
                                stop=sp,
                                perf_mode=DR,
                            )
                        rec = rec_pool.tile([P, 1], f32, tag="rec", name="rec_t")
                        nc.vector.reciprocal(out=rec[:], in_=pd[:])
                        for dc in range(2):
                            sl = slice(dc * NQ, (dc + 1) * NQ)
                            ob = out_pool.tile([P, NQ], f32, tag="ob", name="ob_t")
                            nc.scalar.activation(
                                out=ob[:], in_=po[dc][:], func=AF.Copy,
                                bias=0.0, scale=rec[:],
                            )
                            nc.vector.tensor_add(
                                out=ob[:], in0=ob[:], in1=vres[qg][:, sl]
                            )
                            nc.sync.dma_start(
                                out=out[qg * P : (qg + 1) * P, sl], in_=ob[:]
                            )

    if split_waits:
        _split_excess_waits(nc)
    return nc


def _pair_rows(a):
    """[D, C] -> [D//2, 2*C]: row 256j+128i+p lands at row 128j+p, col
    block i (DoubleRow pairing along the contraction dim)."""
    Din, C = a.shape
    return np.ascontiguousarray(
        a.reshape(Din // 256, 2, P, C).transpose(0, 2, 1, 3).reshape(Din // 2, 2 * C)
    )


def _prep_inputs(plms1, plms2, plms3, seqlengths, Wq, bq, Wk, bk, Wv, bv):
    """Host-side shard + layout prep. Returns in_maps for 8 cores."""
    import ml_dtypes

    bf = ml_dtypes.bfloat16
    f8 = ml_dtypes.float8_e4m3
    f32 = np.float32

    def to_f8(a):
        return np.clip(a, -240.0, 240.0).astype(f8)

    Wq, Wk, Wv = np.asarray(Wq, f32), np.asarray(Wk, f32), np.asarray(Wv, f32)
    bq, bk, bv = np.asarray(bq, f32), np.asarray(bk, f32), np.asarray(bv, f32)

    M2 = (Wq.T @ Wk).astype(f32)            # scores = x1 @ M2 @ x2^T + beta
    m2p = _pair_rows(to_f8(M2_SCALE * M2))
    betav = Wk.T @ bq                        # beta_k = x2_k . betav
    wvt = np.ascontiguousarray(Wv.T).astype(bf)
    bvr = bv
    seqlengths = np.asarray(seqlengths)

    in_maps = []
    ar = np.arange(S)
    for b in range(B):
        x1b = np.asarray(plms1[b], f32)
        x2b = np.asarray(plms2[b], f32)
        x3b = np.asarray(plms3[b], f32)
        beta = x2b @ betav                   # [S]
        bias = SCALE * beta + np.where(ar < int(seqlengths[b]), 0.0, NEG_MASK)
        biasp = np.ascontiguousarray(bias.astype(f32).reshape(KT_TILES, P).T)
        in_maps.append(
            {
                "x1p": _pair_rows(to_f8(x1b.T)),
                "x2p": _pair_rows(to_f8(x2b.T)),
                "m2p": m2p,
                "x3t": np.ascontiguousarray(x3b.T).astype(bf),
                "wvt": wvt,
                "bvr": bvr,
                "biasp": biasp,
            }
        )
    return in_maps


def kernel(**inputs) -> np.ndarray:
    from concourse.bass_utils import run_bass_kernel_spmd

    _install_neff_cache()

    in_maps = _prep_inputs(
        inputs["plms1"], inputs["plms2"], inputs["plms3"], inputs["seqlengths"],
        inputs["Wq"], inputs["bq"], inputs["Wk"], inputs["bk"],
        inputs["Wv"], inputs["bv"],
    )
    nc = build_nc()
    res = run_bass_kernel_spmd(nc, in_maps, core_ids=list(range(B)))
    return np.stack(
        [np.asarray(res.results[i]["out"], np.float32) for i in range(B)]
    )
